# revision 46
# baseline (speedup 1.0000x reference)
"""Bass/Tile TRN2 kernel for nn_BigramLanguageModel (8-layer dense transformer).

Strategy: pure data-parallel over batch across the 8 NeuronCores (8 batch
items / core, no collectives). The residual stream is kept feature-major
([C, tokens]) in SBUF so every matmul contracts over the partition dim.

v2 restructure (vs the first working version):
  - Two-stage software pipeline per (layer, token-chunk) step: the entire
    FFN (FFN1+FFN2) of the previous chunk is deferred and issued interleaved
    with the LN/QKV/attention of the current chunk, so the PE always has
    dense matmul work while the attention exp/mask chains drain on the
    scalar/vector engines.
  - Attention: per (batch-item, head) scores -> exp -> diagonal-block-only
    mask (the off-diagonal quadrants need no mask) -> denominator one-hot
    matmuls -> output matmuls packed 4 heads per PSUM bank. The softmax
    normalization is fused into the PSUM->SBUF evacuation (one DVE multiply
    per head-quad), removing the scalar-engine copies entirely.
  - LN A/B rows are broadcast by K=1 matmuls then copied to SBUF so only
    one PSUM bank is held transiently; evac/apply work is split across
    DVE / Pool / ACT to balance engine load.
  - LM head computed output-transposed ([V, tokens], fp32r full rate), with
    the final transpose done on host during unsharding.
  - Per-layer weights double-buffered so layer l+1's DMA overlaps layer l.
"""

import os
import sys
from contextlib import ExitStack

import numpy as np

for _p in ("/opt/trn_rl_repo", "/root/.axon_site/_ro/trn_rl_repo"):
    if os.path.isdir(_p) and _p not in sys.path:
        sys.path.insert(0, _p)
        break

import concourse.bass as bass
import concourse.mybir as mybir
import concourse.tile as tile
from concourse import bacc

# model config (hardcoded per problem spec)
B, T, C, H, L, V = 64, 256, 512, 8, 8, 100
HD = C // H          # 64
FF = 4 * C           # 2048
EPS = 1e-5
NCORES = 8
BL = B // NCORES     # 8 batch items per core
NT = BL * T          # 2048 tokens per core
P = 128
NCC = C // P         # 4 c-chunks
NFF = FF // P        # 16 ff-chunks
TCH = 512            # token chunk (2 batch items)
NTC = NT // TCH      # 4
BI = TCH // T        # 2 batch items per token chunk

F32 = mybir.dt.float32
F16 = mybir.dt.float16
F32R = mybir.dt.float32r
ADD = mybir.AluOpType.add
MULT = mybir.AluOpType.mult
SUB = mybir.AluOpType.subtract
AF = mybir.ActivationFunctionType


def _r(ap):
    """view an fp32 AP as float32r for full-rate full-precision matmul"""
    return ap if ap.dtype == F32R else ap.bitcast(F32R)


def build_bass_v2():
    """zero-bias build (the actual problem instance: all biases are zero,
    ln gains are folded into the weights on the host)."""
    nc = bacc.Bacc()
    dp = nc.declare_dram_parameter

    onehot_d = dp("onehotT", [V, NT], F16, False)
    tok_d = dp("tok_emb16", [V, C], F16, False)
    pos2_d = dp("pos2T16", [C, TCH], F16, False)
    tri_d = dp("triT", [P, P], F16, False)
    wq_d = dp("wq", [L, C, C], F16, False)
    wk_d = dp("wk", [L, C, C], F16, False)
    wv_d = dp("wv", [L, C, C], F16, False)
    wo_d = dp("wo", [L, C, C], F16, False)
    w1_d = dp("w1", [L, C, FF], F16, False)
    w2_d = dp("w2", [L, FF, C], F16, False)
    e8_d = dp("e8sel", [NCC, H, P], F32, False)
    wlm_d = dp("wlm", [C, V], F32, False)
    out_d = dp("outT", [V, NT], F32, True)

    with tile.TileContext(nc) as tc, ExitStack() as ctx:
        # ---------------- pools ----------------
        pconst = ctx.enter_context(tc.tile_pool(name="const", bufs=1))
        px = ctx.enter_context(tc.tile_pool(name="x", bufs=1))
        pw = ctx.enter_context(tc.tile_pool(name="w", bufs=2))
        ph = ctx.enter_context(tc.tile_pool(name="h", bufs=1))
        pq = ctx.enter_context(tc.tile_pool(name="q", bufs=1))
        pv = ctx.enter_context(tc.tile_pool(name="v", bufs=1))
        po = ctx.enter_context(tc.tile_pool(name="o", bufs=1))
        pffn = ctx.enter_context(tc.tile_pool(name="ffn", bufs=1))
        psq = ctx.enter_context(tc.tile_pool(name="sq", bufs=2))
        pstat = ctx.enter_context(tc.tile_pool(name="stat", bufs=1))
        pstat2 = ctx.enter_context(tc.tile_pool(name="stat2", bufs=1))
        pe_ = ctx.enter_context(tc.tile_pool(name="e", bufs=5))
        prd = ctx.enter_context(tc.tile_pool(name="rd", bufs=2))
        plog = ctx.enter_context(tc.tile_pool(name="log", bufs=1))
        # PSUM (8 banks): mm ring 3 + sc ring 2 (scores, reused for the
        # normalizer broadcast) + den 1 + o-banks 2
        pmm = ctx.enter_context(tc.tile_pool(name="mm", bufs=3, space="PSUM"))
        psc = ctx.enter_context(tc.tile_pool(name="scps", bufs=2, space="PSUM"))
        pdn = ctx.enter_context(tc.tile_pool(name="dn", bufs=1, space="PSUM"))
        pob = ctx.enter_context(tc.tile_pool(name="ob", bufs=2, space="PSUM"))

        # ---------------- constants ----------------
        ones_f = pconst.tile([P, 1], F32, tag="ones_f", name="ones_f")
        nc.vector.memset(ones_f, 1.0)
        ones = pconst.tile([P, 1], F32R, tag="ones", name="ones")
        nc.vector.tensor_copy(ones, ones_f)
        ones1_f = pconst.tile([1, P], F32, tag="ones1_f", name="ones1_f")
        nc.vector.memset(ones1_f, 1.0)
        ones1 = pconst.tile([1, P], F32R, tag="ones1", name="ones1")
        nc.vector.tensor_copy(ones1, ones1_f)
        eps_t = pconst.tile([1, 1], F32, tag="eps", name="eps")
        nc.vector.memset(eps_t, EPS)
        ones16 = pconst.tile([P, 1], F16, tag="ones16", name="ones16")
        nc.vector.memset(ones16, 1.0)
        tri = pconst.tile([P, P], F16, tag="tri", name="tri")
        nc.sync.dma_start(out=tri, in_=tri_d[:, :])
        tok_sb = pconst.tile([V, C], F16, tag="tok", name="tok")
        nc.sync.dma_start(out=tok_sb, in_=tok_d[:, :])
        wlm_sb = []
        for cc in range(NCC):
            f = pconst.tile([P, V], F32, tag=f"wlmf{cc}", name=f"wlmf{cc}")
            nc.sync.dma_start(out=f, in_=wlm_d[cc * P:(cc + 1) * P, :])
            t = pconst.tile([P, V], F32R, tag=f"wlm{cc}", name=f"wlm{cc}")
            nc.vector.tensor_copy(t, f)
            wlm_sb.append(t)
        e8 = []
        for hq in range(NCC):
            f = pconst.tile([H, P], F32, tag=f"e8f{hq}", name=f"e8f{hq}")
            nc.sync.dma_start(out=f, in_=e8_d[hq])
            r8 = pconst.tile([H, P], F32R, tag=f"e8{hq}", name=f"e8{hq}")
            nc.vector.tensor_copy(r8, f)
            e8.append(r8)
        # one-hot columns for denominator matmuls: o8c[p, hh, j] = (j == hh)
        o8c = pconst.tile([P, H, H], F16, tag="o8c", name="o8c")
        nc.vector.memset(o8c, 0.0)
        for hh in range(H):
            nc.vector.memset(o8c[:, hh, hh:hh + 1], 1.0)

        # resident residual stream, feature-major: x_T[c, t]
        x_sb = [px.tile([P, NT], F32R, tag=f"x{cc}", name=f"x{cc}")
                for cc in range(NCC)]

        # ---------------- embedding (staged in the idle ffn1 pool) --------
        pos_sb = []
        for cc in range(NCC):
            t = pffn.tile([P, TCH], F16, tag=f"f{cc}", name=f"pos{cc}")
            nc.sync.dma_start(out=t, in_=pos2_d[cc * P:(cc + 1) * P, :])
            pos_sb.append(t)
        for ti in range(NTC):
            tsl = slice(ti * TCH, (ti + 1) * TCH)
            oh_sb = pffn.tile([V, TCH], F16, tag=f"f{4 + ti}", name="oh")
            nc.sync.dma_start(out=oh_sb, in_=onehot_d[:, tsl])
            for cc in range(NCC):
                ps = pmm.tile([P, TCH], F32, tag="mm", name="mmps")
                nc.tensor.matmul(ps, tok_sb[:, cc * P:(cc + 1) * P],
                                 oh_sb[:, :], start=True, stop=True)
                nc.vector.tensor_add(x_sb[cc][:, tsl], ps, pos_sb[cc])

        # ---------------- per-layer weights (double-buffered) ----------------
        def load_weights(l):
            def _load(dram, tag, n, width):
                ts_ = []
                for i in range(n):
                    t = pw.tile([P, width], F16, tag=f"{tag}{i}", name=f"{tag}{i}")
                    nc.sync.dma_start(out=t, in_=dram[l, i * P:(i + 1) * P, :])
                    ts_.append(t)
                return ts_

            w = {}
            w["wq"] = _load(wq_d, "wq", NCC, C)
            w["wk"] = _load(wk_d, "wk", NCC, C)
            w["wv"] = _load(wv_d, "wv", NCC, C)
            w["wo"] = _load(wo_d, "wo", NCC, C)
            w["w1"] = _load(w1_d, "w1", NCC, FF)
            w["w2"] = _load(w2_d, "w2", NFF, C)
            return w

        # ---------------- LN: stats + A/B rows in SBUF ----------------
        def ln_chain(ti, which):
            """stats + finish; returns (r_t, b_t) SBUF rows"""
            tsl = slice(ti * TCH, (ti + 1) * TCH)
            # squares on Pool (gpsimd), stats matmuls on PE
            S = psc.tile([33, TCH], F32, tag="scps", name=f"S{which}")
            sqs = []
            for cc in range(NCC):
                sq = psq.tile([P, TCH], F16, tag="sq", name="sq")
                eng = nc.vector if cc < 2 else nc.gpsimd
                eng.tensor_mul(sq, x_sb[cc][:, tsl], x_sb[cc][:, tsl])
                sqs.append(sq)
            for cc in range(NCC):
                nc.tensor.matmul(S[0:1, :], ones[:, :], x_sb[cc][:, tsl],
                                 start=(cc == 0), stop=False)
            for cc in range(NCC):
                nc.tensor.matmul(S[32:33, :], ones16[:, :], sqs[cc][:, :],
                                 start=(cc == 0), stop=(cc == NCC - 1),
                                 tile_position=(0, 32))
            # finish on DVE (+ACT rsqrt): A=rstd, Bn=-m (so B=-m*A)
            m2_t = pstat.tile([1, TCH], F32, tag="m2", name="m2_t")
            v_t = pstat.tile([1, TCH], F32, tag="v", name="v_t")
            pst = pstat2 if which == 0 else pstat
            r_t = pst.tile([1, TCH], F32R, tag=f"r{which}", name="r_t")
            b_t = pst.tile([1, TCH], F32R, tag=f"b{which}", name="b_t")
            # m2 = C*m^2 = S0^2/C via ACT Square (single PSUM operand)
            nc.scalar.activation(m2_t, S[0:1, :], AF.Square,
                                 scale=float(C ** -0.5))
            nc.vector.scalar_tensor_tensor(v_t, m2_t, -1.0,
                                           S[32:33, :], op0=MULT, op1=ADD)
            nc.scalar.activation(v_t, v_t, AF.Sqrt, bias=eps_t[:, :],
                                 scale=1.0 / C)
            with nc.allow_low_precision("fp32r rstd is fp32-equivalent"):
                nc.vector.reciprocal(r_t, v_t)
            nc.vector.scalar_tensor_tensor(b_t, S[0:1, :], -1.0 / C, r_t,
                                           op0=MULT, op1=MULT)
            return r_t, b_t

        def ln_bcast(rb):
            """broadcast rows to [P, TCH] via K=1 matmuls"""
            r_t, b_t = rb
            a_ps = pdn.tile([P, TCH], F32, tag="den", name="a_ps")
            nc.tensor.matmul(a_ps, ones1[:, :], r_t[:, :], start=True, stop=True)
            b_ps = psc.tile([P, TCH], F32, tag="scps", name="b_ps")
            nc.tensor.matmul(b_ps, ones1[:, :], b_t[:, :], start=True, stop=True)
            return a_ps, b_ps

        def ln_emit(ti, which):
            return ln_bcast(ln_chain(ti, which))

        def ln_apply(ti, AB, htag):
            """h = x*A + B, fp16. LN1 on DVE (PSUM A/B), LN2 on Pool (SBUF)"""
            tsl = slice(ti * TCH, (ti + 1) * TCH)
            A_sb, B_sb = AB
            h = []
            for cc in range(NCC):
                d = ph.tile([P, TCH], F16, tag=f"{htag}{cc}", name=f"h{cc}")
                nc.vector.tensor_mul(d, x_sb[cc][:, tsl], A_sb)
                nc.vector.tensor_add(d, d, B_sb)
                h.append(d)
            return h

        # ---------------- QKV ----------------
        def qkv_emit(ti, wt, h1):
            q_t, k_t = [], []
            for dst, wsb, nm in ((k_t, wt["wk"], "k"), (q_t, wt["wq"], "q")):
                for hq in range(NCC):
                    ps = pmm.tile([P, TCH], F32, tag="mm", name="mmps")
                    for cc in range(NCC):
                        nc.tensor.matmul(ps, wsb[cc][:, hq * P:(hq + 1) * P],
                                         h1[cc][:, :], start=(cc == 0),
                                         stop=(cc == NCC - 1))
                    qt = pq.tile([P, TCH], F16, tag=f"{nm}t{hq}",
                                 name=f"{nm}t{hq}")
                    nc.scalar.copy(qt, ps)
                    dst.append(qt)
            v8 = []
            for tt in range(TCH // P):
                ps = pmm.tile([P, C], F32, tag="mm", name="mmps")
                for cc in range(NCC):
                    nc.tensor.matmul(ps, h1[cc][:, tt * P:(tt + 1) * P],
                                     wt["wv"][cc][:, :], start=(cc == 0),
                                     stop=(cc == NCC - 1))
                vt = pv.tile([P, H, HD], F16, tag=f"v{tt}", name=f"vt{tt}")
                nc.scalar.copy(vt, ps[:].rearrange("p (h d) -> p h d", h=H))
                v8.append(vt)
            return q_t, k_t, v8

        # ---------------- attention ----------------
        def attn_phase1(ti, bi, q_t, k_t, v8):
            """scores/exp/mask/den/o for all 8 heads of one batch item.
            returns (den_ps, ob_tiles)"""
            den_ps = pdn.tile([H, T], F32, tag="den", name="den")
            obs = [pob.tile([P, 2, T], F32, tag="ob", name=f"ob{p_}")
                   for p_ in range(2)]
            TP = T + P
            for hh in range(H):
                hq, hr = divmod(hh, 2)
                rsl = slice(hr * HD, (hr + 1) * HD)
                qsl = q_t[hq][rsl, bi * T:(bi + 1) * T]
                ksl0 = k_t[hq][rsl, bi * T: bi * T + P]
                ksl1 = k_t[hq][rsl, bi * T + P: bi * T + 2 * P]
                # contiguous [P, 384]: cols 0:256 = s-chunk0 scores vs all t,
                # cols 256:384 = s-chunk1 vs t 128:256 -> ONE exp op
                sc_ps = psc.tile([P, TP], F32, tag="scps", name="scps")
                nc.tensor.matmul(sc_ps[:, 0:T], ksl0, qsl,
                                 start=True, stop=True)
                nc.tensor.matmul(sc_ps[:, T:TP], ksl1, qsl[:, P:T],
                                 start=True, stop=True)
                e = pe_.tile([P, TP], F16, tag="e", name="e")
                nc.scalar.activation(e[:, :], sc_ps[:, :], AF.Exp)
                # mask only the two diagonal [P,P] blocks (fp16 2x on DVE)
                nc.vector.tensor_mul(e[:, 0:P], e[:, 0:P], tri)
                nc.vector.tensor_mul(e[:, T:TP], e[:, T:TP], tri)
                nc.tensor.matmul(den_ps, o8c[:, hh, :], e[:, 0:T],
                                 start=(hh == 0), stop=False)
                nc.tensor.matmul(den_ps[:, P:T], o8c[:, hh, :], e[:, T:TP],
                                 start=False, stop=(hh == H - 1))
                ob = obs[hq // 2]
                qi = hq % 2
                nc.tensor.matmul(ob[rsl, qi, :], v8[bi * 2][:, hh, :],
                                 e[:, 0:T], start=True, stop=False)
                nc.tensor.matmul(ob[rsl, qi, P:T], v8[bi * 2 + 1][:, hh, :],
                                 e[:, T:TP], start=False, stop=True)
            return den_ps, obs

        def attn_phase2(ti, bi, den_ps, obs, o_t):
            """1/den, broadcast, fused normalize-evacuation. A TensorTensor
            may read only one PSUM operand, so rdb is staged to SBUF."""
            rden = prd.tile([H, T], F32R, tag="rden", name="rden")
            with nc.allow_low_precision("fp32r rden is fp32-equivalent"):
                nc.vector.reciprocal(rden, den_ps)
            for p_ in range(2):
                rdb = pdn.tile([P, 2, T], F32, tag="den", name="rdb")
                nc.tensor.matmul(rdb[:, 0, :], e8[2 * p_][:, :], rden[:, :],
                                 start=True, stop=True)
                nc.tensor.matmul(rdb[:, 1, :], e8[2 * p_ + 1][:, :], rden[:, :],
                                 start=True, stop=True)
                rdb_sb = prd.tile([P, 2, T], F16, tag=f"rdb{p_}",
                                  name=f"rdb{p_}")
                nc.scalar.copy(rdb_sb, rdb)
                nc.vector.tensor_mul(
                    o_t[:, 2 * p_:2 * p_ + 2, bi * T:(bi + 1) * T],
                    obs[p_][:, :, :], rdb_sb[:, :, :])

        # ---------------- proj / FFN ----------------
        def proj_emit(ti, wt, o_t, bi):
            w = T
            for cc in range(NCC):
                tsl = slice(ti * TCH + bi * w, ti * TCH + (bi + 1) * w)
                fsl = slice(bi * w, (bi + 1) * w)
                ps = pmm.tile([P, w], F32, tag="mm", name="mmps")
                for hq in range(NCC):
                    nc.tensor.matmul(ps, wt["wo"][hq][:, cc * P:(cc + 1) * P],
                                     o_t[:, hq, fsl], start=(hq == 0),
                                     stop=(hq == NCC - 1))
                nc.vector.tensor_add(x_sb[cc][:, tsl], ps, x_sb[cc][:, tsl])

        def ffn1_emit(wt, h2, fcs, alt_pool=False):
            out = []
            for fc in fcs:
                if alt_pool and fc % 2 == 0:
                    ps = pob.tile([P, TCH], F32, tag="ob", name="obps")
                else:
                    ps = pmm.tile([P, TCH], F32, tag="mm", name="mmps")
                for cc in range(NCC):
                    nc.tensor.matmul(ps, wt["w1"][cc][:, fc * P:(fc + 1) * P],
                                     h2[cc][:, :], start=(cc == 0),
                                     stop=(cc == NCC - 1))
                ft = pffn.tile([P, TCH], F16, tag=f"f{fc}", name=f"ft{fc}")
                if fc % 4 != 3:
                    nc.scalar.activation(ft, ps, AF.Relu)
                else:
                    nc.vector.tensor_scalar_max(ft, ps, 0.0)
                out.append(ft)
            return out

        def ffn2_emit(wt, ffn1, ti, ccs, alt_pool=False, halves=False):
            for cc in ccs:
                tks = ((0, 1) if halves else (None,))
                for tk in tks:
                    if tk is None:
                        w = TCH
                        tsl = slice(ti * TCH, (ti + 1) * TCH)
                        fsl = slice(0, TCH)
                    else:
                        w = TCH // 2
                        tsl = slice(ti * TCH + tk * w, ti * TCH + (tk + 1) * w)
                        fsl = slice(tk * w, (tk + 1) * w)
                    if alt_pool:
                        ps = pob.tile([P, w], F32, tag="ob", name="obps")
                    else:
                        ps = pmm.tile([P, w], F32, tag="mm", name="mmps")
                    for fc in range(NFF):
                        nc.tensor.matmul(ps,
                                         wt["w2"][fc][:, cc * P:(cc + 1) * P],
                                         ffn1[fc][:, fsl], start=(fc == 0),
                                         stop=(fc == NFF - 1))
                    nc.vector.tensor_add(x_sb[cc][:, tsl], ps,
                                         x_sb[cc][:, tsl])

        # ---------------- lm head (transposed out) ----------------
        def lm_emit(ti):
            tsl = slice(ti * TCH, (ti + 1) * TCH)
            ps = pmm.tile([V, TCH], F32, tag="mm", name="lmps")
            for cc in range(NCC):
                nc.tensor.matmul(ps, wlm_sb[cc][:, :], x_sb[cc][:, tsl],
                                 start=(cc == 0), stop=(cc == NCC - 1))
            lo = plog.tile([V, TCH], F32, tag="lg", name="lo")
            nc.scalar.copy(lo, ps)
            nc.sync.dma_start(out=out_d[:, tsl], in_=lo)

        # ---------------- main pipeline (2-deep) ----------------
        # pend1: (wt, h2, ti, layer) -> FFN1 + FFN2[0,1] run next step
        # pend2: (wt, f1, ti, layer) -> FFN2[2,3] run the step after
        pend1 = None
        pend2 = None
        wt = load_weights(0)
        for l in range(L):
            wt_next = load_weights(l + 1) if l + 1 < L else None
            for ti in range(NTC):
                # boundary filler: finish the 2-step-old chunk's FFN2
                if pend2 is not None:
                    ffn2_emit(pend2[0], pend2[1], pend2[2], [0], alt_pool=True)
                AB1 = ln_emit(ti, 0)
                if pend2 is not None:
                    ffn2_emit(pend2[0], pend2[1], pend2[2], [1], alt_pool=True)
                    if pend2[3] == L - 1:
                        lm_emit(pend2[2])
                    pend2 = None
                if pend1 is not None:
                    f1 = ffn1_emit(pend1[0], pend1[1], range(0, NFF // 2),
                                   alt_pool=True)
                h1 = ln_apply(ti, AB1, "h")
                if pend1 is not None:
                    f1 += ffn1_emit(pend1[0], pend1[1], range(NFF // 2, NFF))
                q_t, k_t, v8 = qkv_emit(ti, wt, h1)
                den0, obs0 = attn_phase1(ti, 0, q_t, k_t, v8)
                if pend1 is not None:
                    ffn2_emit(pend1[0], f1, pend1[2], [2], halves=True)
                o_t = po.tile([P, NCC, TCH], F16, tag="ot", name="ot")
                attn_phase2(ti, 0, den0, obs0, o_t)
                den1, obs1 = attn_phase1(ti, 1, q_t, k_t, v8)
                proj_emit(ti, wt, o_t, 0)
                attn_phase2(ti, 1, den1, obs1, o_t)
                proj_emit(ti, wt, o_t, 1)
                AB2 = ln_emit(ti, 1)
                if pend1 is not None:
                    ffn2_emit(pend1[0], f1, pend1[2], [3], halves=True)
                h2 = ln_apply(ti, AB2, "g")
                if pend1 is not None:
                    pend2 = (pend1[0], f1, pend1[2], pend1[3])
                pend1 = (wt, h2, ti, l)
            wt = wt_next if wt_next is not None else wt

        # epilogue
        if pend2 is not None:
            ffn2_emit(pend2[0], pend2[1], pend2[2], [0, 1])
            if pend2[3] == L - 1:
                lm_emit(pend2[2])
        f1 = ffn1_emit(pend1[0], pend1[1], range(NFF))
        ffn2_emit(pend1[0], f1, pend1[2], [0, 1, 2, 3])
        lm_emit(pend1[2])

    if not nc.is_finalized():
        nc.finalize()
    return nc


# ---------------------------------------------------------------------------
# legacy generic build (supports non-zero biases; kept as fallback)
# ---------------------------------------------------------------------------
def _bcast_dram(vec_ap, parts):
    return bass.AP(
        tensor=vec_ap.tensor,
        offset=vec_ap.offset,
        ap=[[0, parts]] + [list(d) for d in vec_ap.ap],
    )


def build_bass_legacy(zero_attn_bias=False, zero_mlp_bias=False):
    nc = bacc.Bacc()
    dp = nc.declare_dram_parameter

    onehot_d = dp("onehotT", [V, NT], F16, False)
    tok_d = dp("tok_emb16", [V, C], F16, False)
    pos2_d = dp("pos2T", [C, TCH], F32, False)
    mask_d = dp("maskT", [P, 2, T], F16, False)
    wq_d = dp("wq", [L, C, C], F16, False)
    wk_d = dp("wk", [L, C, C], F16, False)
    wv_d = dp("wv", [L, C, C], F16, False)
    wo_d = dp("wo", [L, C, C], F16, False)
    w1_d = dp("w1", [L, C, FF], F16, False)
    w2_d = dp("w2", [L, FF, C], F16, False)
    bq_d = dp("bq", [L, C], F32, False)
    bk_d = dp("bk", [L, C], F32, False)
    bv_d = dp("bv", [L, C], F32, False)
    bo_d = dp("bo", [L, C], F32, False)
    b1_d = dp("b1", [L, FF], F32, False)
    b2_d = dp("b2", [L, C], F32, False)
    e8_d = dp("e8sel", [NCC, H, P], F32, False)
    wlm_d = dp("wlm", [C, V], F32, False)
    blm_d = dp("blm", [V], F32, False)
    out_d = dp("out", [NT, V], F32, True)

    with tile.TileContext(nc) as tc, ExitStack() as ctx:
        pconst = ctx.enter_context(tc.tile_pool(name="const", bufs=1))
        px = ctx.enter_context(tc.tile_pool(name="x", bufs=1))
        pw = ctx.enter_context(tc.tile_pool(name="w", bufs=1))
        pbias = ctx.enter_context(tc.tile_pool(name="bias", bufs=1))
        ph = ctx.enter_context(tc.tile_pool(name="h", bufs=2))
        pq = ctx.enter_context(tc.tile_pool(name="q", bufs=1))
        pv = ctx.enter_context(tc.tile_pool(name="v", bufs=2))
        po = ctx.enter_context(tc.tile_pool(name="o", bufs=1))
        pffn = ctx.enter_context(tc.tile_pool(name="ffn", bufs=1))
        psq = ctx.enter_context(tc.tile_pool(name="sq", bufs=2))
        pstat = ctx.enter_context(tc.tile_pool(name="stat", bufs=2))
        pe_ = ctx.enter_context(tc.tile_pool(name="e", bufs=6))
        prd = ctx.enter_context(tc.tile_pool(name="rd", bufs=4))
        plog = ctx.enter_context(tc.tile_pool(name="log", bufs=2))
        pmm = ctx.enter_context(tc.tile_pool(name="mm", bufs=4, space="PSUM"))
        psc = ctx.enter_context(tc.tile_pool(name="scps", bufs=2, space="PSUM"))
        pops = ctx.enter_context(tc.tile_pool(name="ops", bufs=2, space="PSUM"))

        ones_f = pconst.tile([P, 1], F32, tag="ones_f", name="ones_f")
        nc.vector.memset(ones_f, 1.0)
        ones = pconst.tile([P, 1], F32R, tag="ones", name="ones")
        nc.vector.tensor_copy(ones, ones_f)
        ones1_f = pconst.tile([1, P], F32, tag="ones1_f", name="ones1_f")
        nc.vector.memset(ones1_f, 1.0)
        ones1 = pconst.tile([1, P], F32R, tag="ones1", name="ones1")
        nc.vector.tensor_copy(ones1, ones1_f)
        eps_t = pconst.tile([1, 1], F32, tag="eps", name="eps")
        nc.vector.memset(eps_t, EPS)
        mask_sb = pconst.tile([P, 2, T], F16, tag="mask", name="mask")
        nc.sync.dma_start(out=mask_sb, in_=mask_d[:, :, :])
        tok_sb = pconst.tile([V, C], F16, tag="tok", name="tok")
        nc.sync.dma_start(out=tok_sb, in_=tok_d[:, :])
        wlm_sb = []
        for cc in range(NCC):
            t = pconst.tile([P, V], F32, tag=f"wlm{cc}", name=f"wlm{cc}")
            nc.sync.dma_start(out=t, in_=wlm_d[cc * P:(cc + 1) * P, :])
            wlm_sb.append(t)
        blm_bc = pconst.tile([P, V], F32, tag="blm", name="blm")
        nc.sync.dma_start(out=blm_bc, in_=_bcast_dram(blm_d[:], P))
        e8 = []
        for hq in range(NCC):
            f = pconst.tile([H, P], F32, tag=f"e8f{hq}", name=f"e8f{hq}")
            nc.sync.dma_start(out=f, in_=e8_d[hq])
            r8 = pconst.tile([H, P], F32R, tag=f"e8{hq}", name=f"e8{hq}")
            nc.vector.tensor_copy(r8, f)
            e8.append(r8)
        o8c = pconst.tile([P, H, H], F16, tag="o8c", name="o8c")
        nc.vector.memset(o8c, 0.0)
        for hh in range(H):
            nc.vector.memset(o8c[:, hh, hh:hh + 1], 1.0)

        x_sb = [px.tile([P, NT], F32R, tag=f"x{cc}", name=f"x{cc}")
                for cc in range(NCC)]

        with tc.tile_pool(name="emb", bufs=1) as pemb:
            oh_sb = pemb.tile([V, NT], F16, tag="oh", name="oh")
            nc.sync.dma_start(out=oh_sb, in_=onehot_d[:, :])
            pos_sb = []
            for cc in range(NCC):
                t = pemb.tile([P, TCH], F32, tag=f"pos{cc}", name=f"pos{cc}")
                nc.sync.dma_start(out=t, in_=pos2_d[cc * P:(cc + 1) * P, :])
                pos_sb.append(t)
            for ti in range(NTC):
                tsl = slice(ti * TCH, (ti + 1) * TCH)
                for cc in range(NCC):
                    ps = pmm.tile([P, TCH], F32, tag="mm", name="mmps")
                    nc.tensor.matmul(ps, tok_sb[:, cc * P:(cc + 1) * P],
                                     oh_sb[:, tsl], start=True, stop=True)
                    nc.vector.tensor_add(x_sb[cc][:, tsl], ps, pos_sb[cc])

        def ln_stats(tsl):
            S0 = psc.tile([1, TCH], F32, tag="scps", name="S0")
            S1 = psc.tile([1, TCH], F32, tag="scps", name="S1")
            for cc in range(NCC):
                sq = psq.tile([P, TCH], F32R, tag="sq", name="sq")
                nc.vector.tensor_mul(sq, x_sb[cc][:, tsl], x_sb[cc][:, tsl])
                nc.tensor.matmul(S0[0:1, :], _r(ones[:, :]), x_sb[cc][:, tsl],
                                 start=(cc == 0), stop=(cc == NCC - 1))
                nc.tensor.matmul(S1[0:1, :], _r(ones[:, :]), sq[:, :],
                                 start=(cc == 0), stop=(cc == NCC - 1))
            return S0, S1

        def ln_finish(S0, S1):
            m_t = pstat.tile([1, TCH], F32R, tag="m", name="m_t")
            v_t = pstat.tile([1, TCH], F32R, tag="v", name="v_t")
            m2_t = pstat.tile([1, TCH], F32, tag="m2", name="m2_t")
            nc.vector.tensor_scalar_mul(m_t, S0[0:1, :], 1.0 / C)
            nc.vector.tensor_scalar_mul(v_t, S1[0:1, :], 1.0 / C)
            nc.vector.tensor_mul(m2_t, m_t, m_t)
            nc.vector.tensor_sub(v_t, v_t, m2_t)
            nc.scalar.activation(v_t, v_t, AF.Sqrt, bias=eps_t[:, :], scale=1.0)
            with nc.allow_low_precision("fp32r rstd is fp32-equivalent"):
                nc.vector.reciprocal(v_t, v_t)
            nc.vector.scalar_tensor_tensor(m_t, m_t, -1.0, v_t,
                                           op0=MULT, op1=MULT)
            return v_t, m_t

        def ln_bcast(v_t, m_t):
            a_ps = pmm.tile([P, TCH], F32, tag="mm", name="a_ps")
            nc.tensor.matmul(a_ps, _r(ones1[:, :]), v_t[:, :],
                             start=True, stop=True)
            b_ps = pmm.tile([P, TCH], F32, tag="mm", name="b_ps")
            nc.tensor.matmul(b_ps, _r(ones1[:, :]), m_t[:, :],
                             start=True, stop=True)
            return a_ps, b_ps

        def ln_apply(tsl, a_ps, b_ps, htag):
            h = []
            for cc in range(NCC):
                d = ph.tile([P, TCH], F16, tag=f"{htag}{cc}", name=f"h{cc}")
                nc.vector.tensor_mul(d, x_sb[cc][:, tsl], a_ps)
                nc.vector.tensor_add(d, d, b_ps)
                h.append(d)
            return h

        def load_weights(l):
            def _load(dram, tag, n, width):
                ts_ = []
                for i in range(n):
                    t = pw.tile([P, width], F16, tag=f"{tag}{i}", name=f"{tag}{i}")
                    nc.sync.dma_start(out=t, in_=dram[l, i * P:(i + 1) * P, :])
                    ts_.append(t)
                return ts_

            w = {}
            w["wq"] = _load(wq_d, "wq", NCC, C)
            w["wk"] = _load(wk_d, "wk", NCC, C)
            w["wv"] = _load(wv_d, "wv", NCC, C)
            w["wo"] = _load(wo_d, "wo", NCC, C)
            w["w1"] = _load(w1_d, "w1", NCC, FF)
            w["w2"] = _load(w2_d, "w2", NFF, C)
            if not zero_attn_bias:
                for nm, dr in (("bq", bq_d), ("bk", bk_d), ("bo", bo_d)):
                    t = pbias.tile([P, NCC], F32, tag=nm, name=nm)
                    nc.sync.dma_start(out=t, in_=dr[l].rearrange("(a p) -> p a", p=P))
                    w[nm] = t
                bv_bc = pbias.tile([P, C], F32, tag="bvb", name="bvb")
                nc.sync.dma_start(out=bv_bc, in_=_bcast_dram(bv_d[l], P))
                w["bv_bc"] = bv_bc
            if not zero_mlp_bias:
                t = pbias.tile([P, NFF], F32, tag="b1", name="b1")
                nc.sync.dma_start(out=t, in_=b1_d[l].rearrange("(a p) -> p a", p=P))
                w["b1"] = t
                t = pbias.tile([P, NCC], F32, tag="b2", name="b2")
                nc.sync.dma_start(out=t, in_=b2_d[l].rearrange("(a p) -> p a", p=P))
                w["b2"] = t
            return w

        def ffn2_emit(st, cc_list):
            w2_sb, ffn1, ptsl, wt_ = st
            for cc in cc_list:
                ps = pmm.tile([P, TCH], F32, tag="mm", name="mmps")
                for fc in range(NFF):
                    nc.tensor.matmul(ps, w2_sb[fc][:, cc * P:(cc + 1) * P],
                                     ffn1[fc][:, :], start=(fc == 0),
                                     stop=(fc == NFF - 1))
                if zero_mlp_bias:
                    nc.vector.tensor_add(x_sb[cc][:, ptsl], ps,
                                         x_sb[cc][:, ptsl])
                else:
                    nc.vector.scalar_tensor_tensor(
                        x_sb[cc][:, ptsl], ps, wt_["b2"][:, cc:cc + 1],
                        x_sb[cc][:, ptsl], op0=ADD, op1=ADD)

        pending = None

        for l in range(L):
            wt = load_weights(l)
            for ti in range(NTC):
                tsl = slice(ti * TCH, (ti + 1) * TCH)

                S0, S1 = ln_stats(tsl)
                ABl = ln_finish(S0, S1)
                if pending is not None:
                    ffn2_emit(pending, [0, 1])
                a_ps, b_ps = ln_bcast(*ABl)
                if pending is not None:
                    ffn2_emit(pending, [2, 3])
                    pending = None
                h1 = ln_apply(tsl, a_ps, b_ps, "h")

                q_t, k_t = [], []
                for dst, wsb, bnm in ((q_t, wt["wq"], "bq"),
                                      (k_t, wt["wk"], "bk")):
                    for hq in range(NCC):
                        ps = pmm.tile([P, TCH], F32, tag="mm", name="mmps")
                        for cc in range(NCC):
                            nc.tensor.matmul(ps, wsb[cc][:, hq * P:(hq + 1) * P],
                                             h1[cc][:, :], start=(cc == 0),
                                             stop=(cc == NCC - 1))
                        qt = pq.tile([P, TCH], F16, tag=f"{bnm}t{hq}",
                                     name=f"{bnm}t{hq}")
                        if zero_attn_bias:
                            nc.scalar.copy(qt, ps)
                        else:
                            nc.scalar.activation(qt, ps, AF.Identity,
                                                 bias=wt[bnm][:, hq:hq + 1],
                                                 scale=1.0)
                        dst.append(qt)
                v8 = []
                for tt in range(TCH // P):
                    ps = pmm.tile([P, C], F32, tag="mm", name="mmps")
                    for cc in range(NCC):
                        nc.tensor.matmul(ps, h1[cc][:, tt * P:(tt + 1) * P],
                                         wt["wv"][cc][:, :], start=(cc == 0),
                                         stop=(cc == NCC - 1))
                    vt = pv.tile([P, H, HD], F16, tag=f"v{tt}", name=f"vt{tt}")
                    if zero_attn_bias:
                        nc.scalar.copy(vt, ps[:].rearrange("p (h d) -> p h d", h=H))
                    else:
                        nc.vector.tensor_add(
                            vt, ps[:].rearrange("p (h d) -> p h d", h=H),
                            wt["bv_bc"][:].rearrange("p (h d) -> p h d", h=H))
                    v8.append(vt)

                o_t = [po.tile([P, TCH], F16, tag=f"o{hq}", name=f"ot{hq}")
                       for hq in range(NCC)]
                for bi in range(BI):
                    den_ps = pmm.tile([H, T], F32, tag="mm", name="den_ps")
                    for hh in range(H):
                        hq, hr = divmod(hh, 2)
                        rsl = slice(hr * HD, (hr + 1) * HD)
                        qsl = q_t[hq][rsl, bi * T:(bi + 1) * T]
                        ksl0 = k_t[hq][rsl, bi * T: bi * T + P]
                        ksl1 = k_t[hq][rsl, bi * T + P: bi * T + 2 * P]
                        sc_ps = psc.tile([P, 2, T], F32, tag="scps", name="scps")
                        nc.tensor.matmul(sc_ps[:, 0, :], ksl0, qsl,
                                         start=True, stop=True)
                        nc.tensor.matmul(sc_ps[:, 1, P:T], ksl1, qsl[:, P:T],
                                         start=True, stop=True)
                        e = pe_.tile([P, 2, T], F16, tag="e", name="e")
                        nc.scalar.activation(e[:, 0, :], sc_ps[:, 0, :], AF.Exp)
                        nc.scalar.activation(e[:, 1, P:T], sc_ps[:, 1, P:T],
                                             AF.Exp)
                        nc.gpsimd.tensor_mul(e[:, 0, :], e[:, 0, :],
                                             mask_sb[:, 0, :])
                        nc.gpsimd.tensor_mul(e[:, 1, P:T], e[:, 1, P:T],
                                             mask_sb[:, 1, P:T])
                        o_ps = pops.tile([HD, T], F32, tag="ops", name="ops")
                        nc.tensor.matmul(o_ps, v8[bi * 2][:, hh, :], e[:, 0, :],
                                         start=True, stop=False)
                        nc.tensor.matmul(o_ps[:, P:T], v8[bi * 2 + 1][:, hh, :],
                                         e[:, 1, P:T], start=False, stop=True)
                        nc.tensor.matmul(den_ps, o8c[:, hh, :], e[:, 0, :],
                                         start=(hh == 0), stop=False)
                        nc.tensor.matmul(den_ps[:, P:T], o8c[:, hh, :],
                                         e[:, 1, P:T], start=False,
                                         stop=(hh == H - 1))
                        nc.scalar.copy(o_t[hq][rsl, bi * T:(bi + 1) * T],
                                       o_ps[0:HD, :])
                    rden = prd.tile([H, T], F32R, tag="rden", name="rden")
                    with nc.allow_low_precision("fp32r rden is fp32-equivalent"):
                        nc.vector.reciprocal(rden, den_ps)
                    for hq in range(NCC):
                        rdb = pmm.tile([P, T], F32, tag="mm", name="rdb")
                        nc.tensor.matmul(rdb, e8[hq][:, :], rden[:, :],
                                         start=True, stop=True)
                        osl = o_t[hq][:, bi * T:(bi + 1) * T]
                        nc.vector.tensor_mul(osl, osl, rdb)

                for cc in range(NCC):
                    ps = pmm.tile([P, TCH], F32, tag="mm", name="mmps")
                    for hq in range(NCC):
                        nc.tensor.matmul(ps, wt["wo"][hq][:, cc * P:(cc + 1) * P],
                                         o_t[hq][:, :], start=(hq == 0),
                                         stop=(hq == NCC - 1))
                    if zero_attn_bias:
                        nc.vector.tensor_add(x_sb[cc][:, tsl], ps,
                                             x_sb[cc][:, tsl])
                    else:
                        nc.vector.scalar_tensor_tensor(
                            x_sb[cc][:, tsl], ps, wt["bo"][:, cc:cc + 1],
                            x_sb[cc][:, tsl], op0=ADD, op1=ADD)

                S0, S1 = ln_stats(tsl)
                ABl = ln_finish(S0, S1)
                a_ps, b_ps = ln_bcast(*ABl)
                h2 = ln_apply(tsl, a_ps, b_ps, "g")
                ffn1 = []
                for fc in range(NFF):
                    ps = pmm.tile([P, TCH], F32, tag="mm", name="mmps")
                    for cc in range(NCC):
                        nc.tensor.matmul(ps, wt["w1"][cc][:, fc * P:(fc + 1) * P],
                                         h2[cc][:, :], start=(cc == 0),
                                         stop=(cc == NCC - 1))
                    ft = pffn.tile([P, TCH], F16, tag=f"f{fc}", name=f"ft{fc}")
                    if zero_mlp_bias:
                        nc.scalar.activation(ft, ps, AF.Relu)
                    else:
                        nc.scalar.activation(ft, ps, AF.Relu,
                                             bias=wt["b1"][:, fc:fc + 1],
                                             scale=1.0)
                    ffn1.append(ft)
                pending = (wt["w2"], ffn1, tsl, wt)

        ffn2_emit(pending, [0, 1, 2, 3])
        pending = None

        for tt in range(NT // P):
            ps = pmm.tile([P, V], F32, tag="mm", name="mmps")
            for cc in range(NCC):
                nc.tensor.matmul(ps, x_sb[cc][:, tt * P:(tt + 1) * P].bitcast(F32),
                                 wlm_sb[cc][:, :], start=(cc == 0),
                                 stop=(cc == NCC - 1))
            lo = plog.tile([P, V], F32, tag="lg", name="lo")
            nc.vector.tensor_add(lo, ps, blm_bc)
            nc.sync.dma_start(out=out_d[tt * P:(tt + 1) * P, :], in_=lo)

    if not nc.is_finalized():
        nc.finalize()
    return nc


def prep_inputs(idx, tok_emb, pos_emb, Wq, Wk, Wv, Wo, bo, ln1_g, ln1_b,
                ln2_g, ln2_b, W1, b1, W2, b2, Wlm, blm):
    """host-side: fold LN affines into weights, build per-core input maps"""
    f32 = np.float32
    idx = np.asarray(idx)
    tok_emb = np.asarray(tok_emb, f32)
    pos_emb = np.asarray(pos_emb, f32)
    scale = C ** -0.5

    wq = np.empty((L, C, C), f32)
    wk = np.empty((L, C, C), f32)
    wv = np.empty((L, C, C), f32)
    wo = np.empty((L, C, C), f32)
    w1 = np.empty((L, C, FF), f32)
    w2 = np.empty((L, FF, C), f32)
    bq = np.empty((L, C), f32)
    bk = np.empty((L, C), f32)
    bv = np.empty((L, C), f32)
    b1f = np.empty((L, FF), f32)
    for l in range(L):
        wq_c = np.asarray(Wq[l], f32).transpose(1, 0, 2).reshape(C, C)
        wk_c = np.asarray(Wk[l], f32).transpose(1, 0, 2).reshape(C, C)
        wv_c = np.asarray(Wv[l], f32).transpose(1, 0, 2).reshape(C, C)
        g1 = np.asarray(ln1_g[l], f32)[:, None]
        b1_ = np.asarray(ln1_b[l], f32)
        g2 = np.asarray(ln2_g[l], f32)[:, None]
        b2_ = np.asarray(ln2_b[l], f32)
        wq[l] = g1 * wq_c * scale
        bq[l] = (b1_ @ wq_c) * scale
        wk[l] = g1 * wk_c
        bk[l] = b1_ @ wk_c
        wv[l] = g1 * wv_c
        bv[l] = b1_ @ wv_c
        wo[l] = np.asarray(Wo[l], f32)
        w1[l] = g2 * np.asarray(W1[l], f32)
        b1f[l] = np.asarray(b1[l], f32) + b2_ @ np.asarray(W1[l], f32)
        w2[l] = np.asarray(W2[l], f32)

    bo = np.asarray(bo, f32)
    b2a = np.asarray(b2, f32)

    # legacy full mask in [s%128, s//128, t] layout
    s_g = np.arange(2 * P).reshape(2, P).T
    mask = (s_g[:, :, None] <= np.arange(T)[None, None, :]).astype(np.float16)
    # v2 mask: one diagonal [P,P] triangular block (s_local <= t_local)
    tri = (np.arange(P)[:, None] <= np.arange(P)[None, :]).astype(np.float16)

    pos2 = np.concatenate([pos_emb.T, pos_emb.T], axis=1)  # [C, 512]

    flags = {
        "zero_attn_bias": not (np.any(bq) or np.any(bk) or np.any(bv)
                               or np.any(bo)),
        "zero_mlp_bias": not (np.any(b1f) or np.any(b2a)),
        "zero_lm_bias": not np.any(np.asarray(blm, f32)),
    }

    e8sel = np.zeros((NCC, H, P), f32)
    for hq in range(NCC):
        for p_ in range(P):
            e8sel[hq, 2 * hq + p_ // HD, p_] = 1.0

    shared = {
        "e8sel": e8sel,
        "tok_emb16": tok_emb.astype(np.float16),
        "pos2T": np.ascontiguousarray(pos2, f32),
        "wq": wq.astype(np.float16), "wk": wk.astype(np.float16),
        "wv": wv.astype(np.float16), "wo": wo.astype(np.float16),
        "w1": w1.astype(np.float16), "w2": w2.astype(np.float16),
        "wlm": np.asarray(Wlm, f32),
    }
    if all(flags.values()):
        shared["triT"] = np.ascontiguousarray(tri)
        shared["pos2T16"] = np.ascontiguousarray(pos2).astype(np.float16)
        del shared["pos2T"]
    else:
        shared["maskT"] = np.ascontiguousarray(mask)
        shared["bq"] = bq
        shared["bk"] = bk
        shared["bv"] = bv
        shared["bo"] = bo
        shared["b1"] = b1f
        shared["b2"] = b2a
        shared["blm"] = np.asarray(blm, f32)
    in_maps = []
    vocab = np.arange(V)
    for core in range(NCORES):
        toks = np.asarray(idx[core * BL:(core + 1) * BL]).reshape(-1)
        oh = (vocab[:, None] == toks[None, :]).astype(np.float16)
        m = dict(shared)
        m["onehotT"] = np.ascontiguousarray(oh)
        in_maps.append(m)
    return in_maps, flags


_NC_CACHE = {}


def get_nc(flags=None):
    if flags is None:
        flags = {"zero_attn_bias": True, "zero_mlp_bias": True,
                 "zero_lm_bias": True}
    if all(flags.get(k, False) for k in
           ("zero_attn_bias", "zero_mlp_bias", "zero_lm_bias")):
        key = "v2"
        if key not in _NC_CACHE:
            _NC_CACHE[key] = build_bass_v2()
    else:
        key = (flags["zero_attn_bias"], flags["zero_mlp_bias"])
        if key not in _NC_CACHE:
            _NC_CACHE[key] = build_bass_legacy(
                zero_attn_bias=flags["zero_attn_bias"],
                zero_mlp_bias=flags["zero_mlp_bias"])
    return _NC_CACHE[key]


def run(in_maps, flags=None, trace=False, **kw):
    from concourse.bass_utils import run_bass_kernel_spmd
    nc = get_nc(flags)
    return run_bass_kernel_spmd(nc, in_maps, list(range(NCORES)),
                                trace=trace, **kw)


def kernel(**inputs):
    in_maps, flags = prep_inputs(**inputs)
    res = run(in_maps, flags)
    outs = []
    for i in range(NCORES):
        r = res.results[i]
        if "outT" in r:
            outs.append(np.ascontiguousarray(r["outT"].T).reshape(BL, T, V))
        else:
            outs.append(r["out"].reshape(BL, T, V))
    return np.concatenate(outs, axis=0).astype(np.float32)


# revision 47
# speedup vs baseline: 1.0630x; 1.0630x over previous
"""Bass/Tile TRN2 kernel for nn_BigramLanguageModel (8-layer dense transformer).

Strategy: pure data-parallel over batch across the 8 NeuronCores (8 batch
items / core, no collectives). The residual stream is kept feature-major
([C, tokens]) in SBUF so every matmul contracts over the partition dim.

v2 restructure (vs the first working version):
  - Two-stage software pipeline per (layer, token-chunk) step: the entire
    FFN (FFN1+FFN2) of the previous chunk is deferred and issued interleaved
    with the LN/QKV/attention of the current chunk, so the PE always has
    dense matmul work while the attention exp/mask chains drain on the
    scalar/vector engines.
  - Attention: per (batch-item, head) scores -> exp -> diagonal-block-only
    mask (the off-diagonal quadrants need no mask) -> denominator one-hot
    matmuls -> output matmuls packed 4 heads per PSUM bank. The softmax
    normalization is fused into the PSUM->SBUF evacuation (one DVE multiply
    per head-quad), removing the scalar-engine copies entirely.
  - LN A/B rows are broadcast by K=1 matmuls then copied to SBUF so only
    one PSUM bank is held transiently; evac/apply work is split across
    DVE / Pool / ACT to balance engine load.
  - LM head computed output-transposed ([V, tokens], fp32r full rate), with
    the final transpose done on host during unsharding.
  - Per-layer weights double-buffered so layer l+1's DMA overlaps layer l.
"""

import os
import sys
from contextlib import ExitStack

import numpy as np

for _p in ("/opt/trn_rl_repo", "/root/.axon_site/_ro/trn_rl_repo"):
    if os.path.isdir(_p) and _p not in sys.path:
        sys.path.insert(0, _p)
        break

import concourse.bass as bass
import concourse.mybir as mybir
import concourse.tile as tile
from concourse import bacc

# model config (hardcoded per problem spec)
B, T, C, H, L, V = 64, 256, 512, 8, 8, 100
HD = C // H          # 64
FF = 4 * C           # 2048
EPS = 1e-5
NCORES = 8
BL = B // NCORES     # 8 batch items per core
NT = BL * T          # 2048 tokens per core
P = 128
NCC = C // P         # 4 c-chunks
NFF = FF // P        # 16 ff-chunks
TCH = 512            # token chunk (2 batch items)
NTC = NT // TCH      # 4
BI = TCH // T        # 2 batch items per token chunk

F32 = mybir.dt.float32
F16 = mybir.dt.float16
F32R = mybir.dt.float32r
ADD = mybir.AluOpType.add
MULT = mybir.AluOpType.mult
SUB = mybir.AluOpType.subtract
AF = mybir.ActivationFunctionType


def _r(ap):
    """view an fp32 AP as float32r for full-rate full-precision matmul"""
    return ap if ap.dtype == F32R else ap.bitcast(F32R)


def build_bass_v2():
    """zero-bias build (the actual problem instance: all biases are zero,
    ln gains are folded into the weights on the host)."""
    nc = bacc.Bacc()
    dp = nc.declare_dram_parameter

    onehot_d = dp("onehotT", [V, NT], F16, False)
    tok_d = dp("tok_emb16", [V, C], F16, False)
    pos2_d = dp("pos2T16", [C, TCH], F16, False)
    tri_d = dp("triT", [P, P], F16, False)
    wq_d = dp("wq", [L, C, C], F16, False)
    wk_d = dp("wk", [L, C, C], F16, False)
    wv_d = dp("wv", [L, C, C], F16, False)
    wo_d = dp("wo", [L, C, C], F16, False)
    w1_d = dp("w1", [L, C, FF], F16, False)
    w2_d = dp("w2", [L, FF, C], F16, False)
    e8_d = dp("e8sel", [NCC, H, P], F32, False)
    wlm_d = dp("wlm", [C, V], F32, False)
    out_d = dp("outT", [V, NT], F32, True)

    with tile.TileContext(nc) as tc, ExitStack() as ctx:
        # ---------------- pools ----------------
        pconst = ctx.enter_context(tc.tile_pool(name="const", bufs=1))
        px = ctx.enter_context(tc.tile_pool(name="x", bufs=1))
        pw = ctx.enter_context(tc.tile_pool(name="w", bufs=2))
        ph = ctx.enter_context(tc.tile_pool(name="h", bufs=1))
        pq = ctx.enter_context(tc.tile_pool(name="q", bufs=1))
        pv = ctx.enter_context(tc.tile_pool(name="v", bufs=1))
        po = ctx.enter_context(tc.tile_pool(name="o", bufs=1))
        pffn = ctx.enter_context(tc.tile_pool(name="ffn", bufs=1))
        psq = ctx.enter_context(tc.tile_pool(name="sq", bufs=2))
        pstat = ctx.enter_context(tc.tile_pool(name="stat", bufs=1))
        pstat2 = ctx.enter_context(tc.tile_pool(name="stat2", bufs=1))
        pe_ = ctx.enter_context(tc.tile_pool(name="e", bufs=5))
        prd = ctx.enter_context(tc.tile_pool(name="rd", bufs=2))
        plog = ctx.enter_context(tc.tile_pool(name="log", bufs=1))
        # PSUM (8 banks): mm ring 3 + sc ring 2 (scores, reused for the
        # normalizer broadcast) + den 1 + o-banks 2
        pmm = ctx.enter_context(tc.tile_pool(name="mm", bufs=3, space="PSUM"))
        psc = ctx.enter_context(tc.tile_pool(name="scps", bufs=2, space="PSUM"))
        pdn = ctx.enter_context(tc.tile_pool(name="dn", bufs=1, space="PSUM"))
        pob = ctx.enter_context(tc.tile_pool(name="ob", bufs=2, space="PSUM"))

        # ---------------- constants ----------------
        ones_f = pconst.tile([P, 1], F32, tag="ones_f", name="ones_f")
        nc.vector.memset(ones_f, 1.0)
        ones = pconst.tile([P, 1], F32R, tag="ones", name="ones")
        nc.vector.tensor_copy(ones, ones_f)
        ones1_f = pconst.tile([1, P], F32, tag="ones1_f", name="ones1_f")
        nc.vector.memset(ones1_f, 1.0)
        ones1 = pconst.tile([1, P], F32R, tag="ones1", name="ones1")
        nc.vector.tensor_copy(ones1, ones1_f)
        eps_t = pconst.tile([1, 1], F32, tag="eps", name="eps")
        nc.vector.memset(eps_t, EPS)
        ones16 = pconst.tile([P, 1], F16, tag="ones16", name="ones16")
        nc.vector.memset(ones16, 1.0)
        tri = pconst.tile([P, P], F16, tag="tri", name="tri")
        nc.sync.dma_start(out=tri, in_=tri_d[:, :])
        tok_sb = pconst.tile([V, C], F16, tag="tok", name="tok")
        nc.sync.dma_start(out=tok_sb, in_=tok_d[:, :])
        wlm_sb = []
        for cc in range(NCC):
            f = pconst.tile([P, V], F32, tag=f"wlmf{cc}", name=f"wlmf{cc}")
            nc.sync.dma_start(out=f, in_=wlm_d[cc * P:(cc + 1) * P, :])
            t = pconst.tile([P, V], F32R, tag=f"wlm{cc}", name=f"wlm{cc}")
            nc.vector.tensor_copy(t, f)
            wlm_sb.append(t)
        e8 = []
        for hq in range(NCC):
            f = pconst.tile([H, P], F32, tag=f"e8f{hq}", name=f"e8f{hq}")
            nc.sync.dma_start(out=f, in_=e8_d[hq])
            r8 = pconst.tile([H, P], F32R, tag=f"e8{hq}", name=f"e8{hq}")
            nc.vector.tensor_copy(r8, f)
            e8.append(r8)
        # one-hot columns for denominator matmuls: o8c[p, hh, j] = (j == hh)
        o8c = pconst.tile([P, H, H], F16, tag="o8c", name="o8c")
        nc.vector.memset(o8c, 0.0)
        for hh in range(H):
            nc.vector.memset(o8c[:, hh, hh:hh + 1], 1.0)

        # resident residual stream, feature-major: x_T[c, t]
        x_sb = [px.tile([P, NT], F32R, tag=f"x{cc}", name=f"x{cc}")
                for cc in range(NCC)]

        # ---------------- embedding (staged in the idle ffn1 pool) --------
        pos_sb = []
        for cc in range(NCC):
            t = pffn.tile([P, TCH], F16, tag=f"f{cc}", name=f"pos{cc}")
            nc.sync.dma_start(out=t, in_=pos2_d[cc * P:(cc + 1) * P, :])
            pos_sb.append(t)
        for ti in range(NTC):
            tsl = slice(ti * TCH, (ti + 1) * TCH)
            oh_sb = pffn.tile([V, TCH], F16, tag=f"f{4 + ti}", name="oh")
            nc.sync.dma_start(out=oh_sb, in_=onehot_d[:, tsl])
            for cc in range(NCC):
                ps = pmm.tile([P, TCH], F32, tag="mm", name="mmps")
                nc.tensor.matmul(ps, tok_sb[:, cc * P:(cc + 1) * P],
                                 oh_sb[:, :], start=True, stop=True)
                nc.vector.tensor_add(x_sb[cc][:, tsl], ps, pos_sb[cc])

        # ---------------- per-layer weights (double-buffered) ----------------
        def load_weights(l):
            def _load(dram, tag, n, width):
                ts_ = []
                for i in range(n):
                    t = pw.tile([P, width], F16, tag=f"{tag}{i}", name=f"{tag}{i}")
                    nc.sync.dma_start(out=t, in_=dram[l, i * P:(i + 1) * P, :])
                    ts_.append(t)
                return ts_

            w = {}
            w["wq"] = _load(wq_d, "wq", NCC, C)
            w["wk"] = _load(wk_d, "wk", NCC, C)
            w["wv"] = _load(wv_d, "wv", NCC, C)
            w["wo"] = _load(wo_d, "wo", NCC, C)
            w["w1"] = _load(w1_d, "w1", NCC, FF)
            w["w2"] = _load(w2_d, "w2", NFF, C)
            return w

        # ---------------- LN: stats + A/B rows in SBUF ----------------
        def ln_stats_half(S, ti, half, first):
            """stats MMs over one 256-token half (so LN2 can chase proj)"""
            hw_ = TCH // 2
            tsl = slice(ti * TCH + half * hw_, ti * TCH + (half + 1) * hw_)
            fsl = slice(half * hw_, (half + 1) * hw_)
            sqs = []
            for cc in range(NCC):
                sq = psq.tile([P, hw_], F16, tag="sq", name="sq")
                eng = nc.vector if cc < 2 else nc.gpsimd
                eng.tensor_mul(sq, x_sb[cc][:, tsl], x_sb[cc][:, tsl])
                sqs.append(sq)
            for cc in range(NCC):
                nc.tensor.matmul(S[0:1, fsl], ones[:, :], x_sb[cc][:, tsl],
                                 start=(first and cc == 0), stop=False)
            for cc in range(NCC):
                nc.tensor.matmul(S[32:33, fsl], ones16[:, :], sqs[cc][:, :],
                                 start=(first and cc == 0),
                                 stop=(cc == NCC - 1),
                                 tile_position=(0, 32))

        def ln_chain(ti, which, S=None):
            """stats + finish; returns (r_t, b_t) SBUF rows"""
            tsl = slice(ti * TCH, (ti + 1) * TCH)
            # squares on Pool (gpsimd), stats matmuls on PE
            if S is None:
                S = psc.tile([33, TCH], F32, tag="scps", name=f"S{which}")
                sqs = []
                for cc in range(NCC):
                    sq = psq.tile([P, TCH], F16, tag="sq", name="sq")
                    eng = nc.vector if cc < 2 else nc.gpsimd
                    eng.tensor_mul(sq, x_sb[cc][:, tsl], x_sb[cc][:, tsl])
                    sqs.append(sq)
                for cc in range(NCC):
                    nc.tensor.matmul(S[0:1, :], ones[:, :], x_sb[cc][:, tsl],
                                     start=(cc == 0), stop=False)
                for cc in range(NCC):
                    nc.tensor.matmul(S[32:33, :], ones16[:, :], sqs[cc][:, :],
                                     start=(cc == 0), stop=(cc == NCC - 1),
                                     tile_position=(0, 32))
            # finish on DVE (+ACT rsqrt): A=rstd, Bn=-m (so B=-m*A)
            m2_t = pstat.tile([1, TCH], F32, tag="m2", name="m2_t")
            v_t = pstat.tile([1, TCH], F32, tag="v", name="v_t")
            pst = pstat2 if which == 0 else pstat
            r_t = pst.tile([1, TCH], F32R, tag=f"r{which}", name="r_t")
            b_t = pst.tile([1, TCH], F32R, tag=f"b{which}", name="b_t")
            # m2 = C*m^2 = S0^2/C via ACT Square (single PSUM operand)
            nc.scalar.activation(m2_t, S[0:1, :], AF.Square,
                                 scale=float(C ** -0.5))
            nc.vector.scalar_tensor_tensor(v_t, m2_t, -1.0,
                                           S[32:33, :], op0=MULT, op1=ADD)
            nc.scalar.activation(v_t, v_t, AF.Sqrt, bias=eps_t[:, :],
                                 scale=1.0 / C)
            with nc.allow_low_precision("fp32r rstd is fp32-equivalent"):
                nc.vector.reciprocal(r_t, v_t)
            nc.vector.scalar_tensor_tensor(b_t, S[0:1, :], -1.0 / C, r_t,
                                           op0=MULT, op1=MULT)
            return r_t, b_t

        def ln_bcast(rb):
            """broadcast rows to [P, TCH] via K=1 matmuls"""
            r_t, b_t = rb
            a_ps = pdn.tile([P, TCH], F32, tag="den", name="a_ps")
            nc.tensor.matmul(a_ps, ones1[:, :], r_t[:, :], start=True, stop=True)
            b_ps = psc.tile([P, TCH], F32, tag="scps", name="b_ps")
            nc.tensor.matmul(b_ps, ones1[:, :], b_t[:, :], start=True, stop=True)
            return a_ps, b_ps

        def ln_emit(ti, which):
            return ln_bcast(ln_chain(ti, which))

        def ln_apply(ti, AB, htag):
            """h = x*A + B, fp16. LN1 on DVE (PSUM A/B), LN2 on Pool (SBUF)"""
            tsl = slice(ti * TCH, (ti + 1) * TCH)
            A_sb, B_sb = AB
            h = []
            for cc in range(NCC):
                d = ph.tile([P, TCH], F16, tag=f"{htag}{cc}", name=f"h{cc}")
                nc.vector.tensor_mul(d, x_sb[cc][:, tsl], A_sb)
                nc.vector.tensor_add(d, d, B_sb)
                h.append(d)
            return h

        # ---------------- QKV ----------------
        def qkv_emit(ti, wt, h1):
            q_t, k_t = [], []
            for dst, wsb, nm in ((k_t, wt["wk"], "k"), (q_t, wt["wq"], "q")):
                for hq in range(NCC):
                    ps = pmm.tile([P, TCH], F32, tag="mm", name="mmps")
                    for cc in range(NCC):
                        nc.tensor.matmul(ps, wsb[cc][:, hq * P:(hq + 1) * P],
                                         h1[cc][:, :], start=(cc == 0),
                                         stop=(cc == NCC - 1))
                    qt = pq.tile([P, TCH], F16, tag=f"{nm}t{hq}",
                                 name=f"{nm}t{hq}")
                    nc.scalar.copy(qt, ps)
                    dst.append(qt)
            v8 = []
            for tt in range(TCH // P):
                ps = pmm.tile([P, C], F32, tag="mm", name="mmps")
                for cc in range(NCC):
                    nc.tensor.matmul(ps, h1[cc][:, tt * P:(tt + 1) * P],
                                     wt["wv"][cc][:, :], start=(cc == 0),
                                     stop=(cc == NCC - 1))
                vt = pv.tile([P, H, HD], F16, tag=f"v{tt}", name=f"vt{tt}")
                nc.scalar.copy(vt, ps[:].rearrange("p (h d) -> p h d", h=H))
                v8.append(vt)
            return q_t, k_t, v8

        # ---------------- attention ----------------
        def attn_phase1(ti, bi, q_t, k_t, v8):
            """scores/exp/mask/den/o for all 8 heads of one batch item.
            returns (den_ps, ob_tiles)"""
            den_ps = pdn.tile([H, T], F32, tag="den", name="den")
            obs = [pob.tile([P, 2, T], F32, tag="ob", name=f"ob{p_}")
                   for p_ in range(2)]
            TP = T + P
            for hh in range(H):
                hq, hr = divmod(hh, 2)
                rsl = slice(hr * HD, (hr + 1) * HD)
                qsl = q_t[hq][rsl, bi * T:(bi + 1) * T]
                ksl0 = k_t[hq][rsl, bi * T: bi * T + P]
                ksl1 = k_t[hq][rsl, bi * T + P: bi * T + 2 * P]
                # contiguous [P, 384]: cols 0:256 = s-chunk0 scores vs all t,
                # cols 256:384 = s-chunk1 vs t 128:256 -> ONE exp op
                sc_ps = psc.tile([P, TP], F32, tag="scps", name="scps")
                nc.tensor.matmul(sc_ps[:, 0:T], ksl0, qsl,
                                 start=True, stop=True)
                nc.tensor.matmul(sc_ps[:, T:TP], ksl1, qsl[:, P:T],
                                 start=True, stop=True)
                e = pe_.tile([P, TP], F16, tag="e", name="e")
                nc.scalar.activation(e[:, :], sc_ps[:, :], AF.Exp)
                # mask only the two diagonal [P,P] blocks (fp16 2x on DVE)
                nc.vector.tensor_mul(e[:, 0:P], e[:, 0:P], tri)
                nc.vector.tensor_mul(e[:, T:TP], e[:, T:TP], tri)
                nc.tensor.matmul(den_ps, o8c[:, hh, :], e[:, 0:T],
                                 start=(hh == 0), stop=False)
                nc.tensor.matmul(den_ps[:, P:T], o8c[:, hh, :], e[:, T:TP],
                                 start=False, stop=(hh == H - 1))
                ob = obs[hq // 2]
                qi = hq % 2
                nc.tensor.matmul(ob[rsl, qi, :], v8[bi * 2][:, hh, :],
                                 e[:, 0:T], start=True, stop=False)
                nc.tensor.matmul(ob[rsl, qi, P:T], v8[bi * 2 + 1][:, hh, :],
                                 e[:, T:TP], start=False, stop=True)
            return den_ps, obs

        def attn_phase2(ti, bi, den_ps, obs, o_t):
            """1/den, broadcast, fused normalize-evacuation. A TensorTensor
            may read only one PSUM operand, so rdb is staged to SBUF."""
            rden = prd.tile([H, T], F32R, tag="rden", name="rden")
            with nc.allow_low_precision("fp32r rden is fp32-equivalent"):
                nc.vector.reciprocal(rden, den_ps)
            for p_ in range(2):
                rdb = pdn.tile([P, 2, T], F32, tag="den", name="rdb")
                nc.tensor.matmul(rdb[:, 0, :], e8[2 * p_][:, :], rden[:, :],
                                 start=True, stop=True)
                nc.tensor.matmul(rdb[:, 1, :], e8[2 * p_ + 1][:, :], rden[:, :],
                                 start=True, stop=True)
                rdb_sb = prd.tile([P, 2, T], F16, tag=f"rdb{p_}",
                                  name=f"rdb{p_}")
                nc.scalar.copy(rdb_sb, rdb)
                nc.vector.tensor_mul(
                    o_t[:, 2 * p_:2 * p_ + 2, bi * T:(bi + 1) * T],
                    obs[p_][:, :, :], rdb_sb[:, :, :])

        # ---------------- proj / FFN ----------------
        def proj_emit(ti, wt, o_t, bi):
            w = T
            for cc in range(NCC):
                tsl = slice(ti * TCH + bi * w, ti * TCH + (bi + 1) * w)
                fsl = slice(bi * w, (bi + 1) * w)
                ps = pmm.tile([P, w], F32, tag="mm", name="mmps")
                for hq in range(NCC):
                    nc.tensor.matmul(ps, wt["wo"][hq][:, cc * P:(cc + 1) * P],
                                     o_t[:, hq, fsl], start=(hq == 0),
                                     stop=(hq == NCC - 1))
                nc.vector.tensor_add(x_sb[cc][:, tsl], ps, x_sb[cc][:, tsl])

        def ffn1_emit(wt, h2, fcs, alt_pool=False):
            out = []
            for fc in fcs:
                if alt_pool and fc % 2 == 0:
                    ps = pob.tile([P, TCH], F32, tag="ob", name="obps")
                else:
                    ps = pmm.tile([P, TCH], F32, tag="mm", name="mmps")
                for cc in range(NCC):
                    nc.tensor.matmul(ps, wt["w1"][cc][:, fc * P:(fc + 1) * P],
                                     h2[cc][:, :], start=(cc == 0),
                                     stop=(cc == NCC - 1))
                ft = pffn.tile([P, TCH], F16, tag=f"f{fc}", name=f"ft{fc}")
                if fc % 4 != 3:
                    nc.scalar.activation(ft, ps, AF.Relu)
                else:
                    nc.vector.tensor_scalar_max(ft, ps, 0.0)
                out.append(ft)
            return out

        def ffn2_emit(wt, ffn1, ti, ccs, alt_pool=False, halves=False):
            for cc in ccs:
                tks = ((0, 1) if halves else (None,))
                for tk in tks:
                    if tk is None:
                        w = TCH
                        tsl = slice(ti * TCH, (ti + 1) * TCH)
                        fsl = slice(0, TCH)
                    else:
                        w = TCH // 2
                        tsl = slice(ti * TCH + tk * w, ti * TCH + (tk + 1) * w)
                        fsl = slice(tk * w, (tk + 1) * w)
                    if alt_pool:
                        ps = pob.tile([P, w], F32, tag="ob", name="obps")
                    else:
                        ps = pmm.tile([P, w], F32, tag="mm", name="mmps")
                    for fc in range(NFF):
                        nc.tensor.matmul(ps,
                                         wt["w2"][fc][:, cc * P:(cc + 1) * P],
                                         ffn1[fc][:, fsl], start=(fc == 0),
                                         stop=(fc == NFF - 1))
                    nc.vector.tensor_add(x_sb[cc][:, tsl], ps,
                                         x_sb[cc][:, tsl])

        # ---------------- lm head (transposed out) ----------------
        def lm_emit(ti):
            tsl = slice(ti * TCH, (ti + 1) * TCH)
            ps = pmm.tile([V, TCH], F32, tag="mm", name="lmps")
            for cc in range(NCC):
                nc.tensor.matmul(ps, wlm_sb[cc][:, :], x_sb[cc][:, tsl],
                                 start=(cc == 0), stop=(cc == NCC - 1))
            lo = plog.tile([V, TCH], F32, tag="lg", name="lo")
            nc.scalar.copy(lo, ps)
            nc.sync.dma_start(out=out_d[:, tsl], in_=lo)

        # ---------------- main pipeline (2-deep) ----------------
        # pend1: (wt, h2, ti, layer) -> FFN1 + FFN2[0,1] run next step
        # pend2: (wt, f1, ti, layer) -> FFN2[2,3] run the step after
        pend1 = None
        pend2 = None
        wt = load_weights(0)
        for l in range(L):
            wt_next = load_weights(l + 1) if l + 1 < L else None
            for ti in range(NTC):
                # boundary filler: finish the 2-step-old chunk's FFN2
                if pend2 is not None:
                    ffn2_emit(pend2[0], pend2[1], pend2[2], [0], alt_pool=True)
                AB1 = ln_emit(ti, 0)
                if pend2 is not None:
                    ffn2_emit(pend2[0], pend2[1], pend2[2], [1], alt_pool=True)
                    if pend2[3] == L - 1:
                        lm_emit(pend2[2])
                    pend2 = None
                if pend1 is not None:
                    f1 = ffn1_emit(pend1[0], pend1[1], range(0, NFF // 2),
                                   alt_pool=True)
                h1 = ln_apply(ti, AB1, "h")
                if pend1 is not None:
                    f1 += ffn1_emit(pend1[0], pend1[1], range(NFF // 2, NFF))
                q_t, k_t, v8 = qkv_emit(ti, wt, h1)
                den0, obs0 = attn_phase1(ti, 0, q_t, k_t, v8)
                if pend1 is not None:
                    ffn2_emit(pend1[0], f1, pend1[2], [2], halves=True)
                o_t = po.tile([P, NCC, TCH], F16, tag="ot", name="ot")
                attn_phase2(ti, 0, den0, obs0, o_t)
                den1, obs1 = attn_phase1(ti, 1, q_t, k_t, v8)
                proj_emit(ti, wt, o_t, 0)
                S2 = psc.tile([33, TCH], F32, tag="scps", name="S1")
                ln_stats_half(S2, ti, 0, True)
                attn_phase2(ti, 1, den1, obs1, o_t)
                proj_emit(ti, wt, o_t, 1)
                ln_stats_half(S2, ti, 1, False)
                AB2 = ln_bcast(ln_chain(ti, 1, S=S2))
                if pend1 is not None:
                    ffn2_emit(pend1[0], f1, pend1[2], [3], halves=True)
                h2 = ln_apply(ti, AB2, "g")
                if pend1 is not None:
                    pend2 = (pend1[0], f1, pend1[2], pend1[3])
                pend1 = (wt, h2, ti, l)
            wt = wt_next if wt_next is not None else wt

        # epilogue
        if pend2 is not None:
            ffn2_emit(pend2[0], pend2[1], pend2[2], [0, 1])
            if pend2[3] == L - 1:
                lm_emit(pend2[2])
        f1 = ffn1_emit(pend1[0], pend1[1], range(NFF))
        ffn2_emit(pend1[0], f1, pend1[2], [0, 1, 2, 3])
        lm_emit(pend1[2])

    if not nc.is_finalized():
        nc.finalize()
    return nc


# ---------------------------------------------------------------------------
# legacy generic build (supports non-zero biases; kept as fallback)
# ---------------------------------------------------------------------------
def _bcast_dram(vec_ap, parts):
    return bass.AP(
        tensor=vec_ap.tensor,
        offset=vec_ap.offset,
        ap=[[0, parts]] + [list(d) for d in vec_ap.ap],
    )


def build_bass_legacy(zero_attn_bias=False, zero_mlp_bias=False):
    nc = bacc.Bacc()
    dp = nc.declare_dram_parameter

    onehot_d = dp("onehotT", [V, NT], F16, False)
    tok_d = dp("tok_emb16", [V, C], F16, False)
    pos2_d = dp("pos2T", [C, TCH], F32, False)
    mask_d = dp("maskT", [P, 2, T], F16, False)
    wq_d = dp("wq", [L, C, C], F16, False)
    wk_d = dp("wk", [L, C, C], F16, False)
    wv_d = dp("wv", [L, C, C], F16, False)
    wo_d = dp("wo", [L, C, C], F16, False)
    w1_d = dp("w1", [L, C, FF], F16, False)
    w2_d = dp("w2", [L, FF, C], F16, False)
    bq_d = dp("bq", [L, C], F32, False)
    bk_d = dp("bk", [L, C], F32, False)
    bv_d = dp("bv", [L, C], F32, False)
    bo_d = dp("bo", [L, C], F32, False)
    b1_d = dp("b1", [L, FF], F32, False)
    b2_d = dp("b2", [L, C], F32, False)
    e8_d = dp("e8sel", [NCC, H, P], F32, False)
    wlm_d = dp("wlm", [C, V], F32, False)
    blm_d = dp("blm", [V], F32, False)
    out_d = dp("out", [NT, V], F32, True)

    with tile.TileContext(nc) as tc, ExitStack() as ctx:
        pconst = ctx.enter_context(tc.tile_pool(name="const", bufs=1))
        px = ctx.enter_context(tc.tile_pool(name="x", bufs=1))
        pw = ctx.enter_context(tc.tile_pool(name="w", bufs=1))
        pbias = ctx.enter_context(tc.tile_pool(name="bias", bufs=1))
        ph = ctx.enter_context(tc.tile_pool(name="h", bufs=2))
        pq = ctx.enter_context(tc.tile_pool(name="q", bufs=1))
        pv = ctx.enter_context(tc.tile_pool(name="v", bufs=2))
        po = ctx.enter_context(tc.tile_pool(name="o", bufs=1))
        pffn = ctx.enter_context(tc.tile_pool(name="ffn", bufs=1))
        psq = ctx.enter_context(tc.tile_pool(name="sq", bufs=2))
        pstat = ctx.enter_context(tc.tile_pool(name="stat", bufs=2))
        pe_ = ctx.enter_context(tc.tile_pool(name="e", bufs=6))
        prd = ctx.enter_context(tc.tile_pool(name="rd", bufs=4))
        plog = ctx.enter_context(tc.tile_pool(name="log", bufs=2))
        pmm = ctx.enter_context(tc.tile_pool(name="mm", bufs=4, space="PSUM"))
        psc = ctx.enter_context(tc.tile_pool(name="scps", bufs=2, space="PSUM"))
        pops = ctx.enter_context(tc.tile_pool(name="ops", bufs=2, space="PSUM"))

        ones_f = pconst.tile([P, 1], F32, tag="ones_f", name="ones_f")
        nc.vector.memset(ones_f, 1.0)
        ones = pconst.tile([P, 1], F32R, tag="ones", name="ones")
        nc.vector.tensor_copy(ones, ones_f)
        ones1_f = pconst.tile([1, P], F32, tag="ones1_f", name="ones1_f")
        nc.vector.memset(ones1_f, 1.0)
        ones1 = pconst.tile([1, P], F32R, tag="ones1", name="ones1")
        nc.vector.tensor_copy(ones1, ones1_f)
        eps_t = pconst.tile([1, 1], F32, tag="eps", name="eps")
        nc.vector.memset(eps_t, EPS)
        mask_sb = pconst.tile([P, 2, T], F16, tag="mask", name="mask")
        nc.sync.dma_start(out=mask_sb, in_=mask_d[:, :, :])
        tok_sb = pconst.tile([V, C], F16, tag="tok", name="tok")
        nc.sync.dma_start(out=tok_sb, in_=tok_d[:, :])
        wlm_sb = []
        for cc in range(NCC):
            t = pconst.tile([P, V], F32, tag=f"wlm{cc}", name=f"wlm{cc}")
            nc.sync.dma_start(out=t, in_=wlm_d[cc * P:(cc + 1) * P, :])
            wlm_sb.append(t)
        blm_bc = pconst.tile([P, V], F32, tag="blm", name="blm")
        nc.sync.dma_start(out=blm_bc, in_=_bcast_dram(blm_d[:], P))
        e8 = []
        for hq in range(NCC):
            f = pconst.tile([H, P], F32, tag=f"e8f{hq}", name=f"e8f{hq}")
            nc.sync.dma_start(out=f, in_=e8_d[hq])
            r8 = pconst.tile([H, P], F32R, tag=f"e8{hq}", name=f"e8{hq}")
            nc.vector.tensor_copy(r8, f)
            e8.append(r8)
        o8c = pconst.tile([P, H, H], F16, tag="o8c", name="o8c")
        nc.vector.memset(o8c, 0.0)
        for hh in range(H):
            nc.vector.memset(o8c[:, hh, hh:hh + 1], 1.0)

        x_sb = [px.tile([P, NT], F32R, tag=f"x{cc}", name=f"x{cc}")
                for cc in range(NCC)]

        with tc.tile_pool(name="emb", bufs=1) as pemb:
            oh_sb = pemb.tile([V, NT], F16, tag="oh", name="oh")
            nc.sync.dma_start(out=oh_sb, in_=onehot_d[:, :])
            pos_sb = []
            for cc in range(NCC):
                t = pemb.tile([P, TCH], F32, tag=f"pos{cc}", name=f"pos{cc}")
                nc.sync.dma_start(out=t, in_=pos2_d[cc * P:(cc + 1) * P, :])
                pos_sb.append(t)
            for ti in range(NTC):
                tsl = slice(ti * TCH, (ti + 1) * TCH)
                for cc in range(NCC):
                    ps = pmm.tile([P, TCH], F32, tag="mm", name="mmps")
                    nc.tensor.matmul(ps, tok_sb[:, cc * P:(cc + 1) * P],
                                     oh_sb[:, tsl], start=True, stop=True)
                    nc.vector.tensor_add(x_sb[cc][:, tsl], ps, pos_sb[cc])

        def ln_stats(tsl):
            S0 = psc.tile([1, TCH], F32, tag="scps", name="S0")
            S1 = psc.tile([1, TCH], F32, tag="scps", name="S1")
            for cc in range(NCC):
                sq = psq.tile([P, TCH], F32R, tag="sq", name="sq")
                nc.vector.tensor_mul(sq, x_sb[cc][:, tsl], x_sb[cc][:, tsl])
                nc.tensor.matmul(S0[0:1, :], _r(ones[:, :]), x_sb[cc][:, tsl],
                                 start=(cc == 0), stop=(cc == NCC - 1))
                nc.tensor.matmul(S1[0:1, :], _r(ones[:, :]), sq[:, :],
                                 start=(cc == 0), stop=(cc == NCC - 1))
            return S0, S1

        def ln_finish(S0, S1):
            m_t = pstat.tile([1, TCH], F32R, tag="m", name="m_t")
            v_t = pstat.tile([1, TCH], F32R, tag="v", name="v_t")
            m2_t = pstat.tile([1, TCH], F32, tag="m2", name="m2_t")
            nc.vector.tensor_scalar_mul(m_t, S0[0:1, :], 1.0 / C)
            nc.vector.tensor_scalar_mul(v_t, S1[0:1, :], 1.0 / C)
            nc.vector.tensor_mul(m2_t, m_t, m_t)
            nc.vector.tensor_sub(v_t, v_t, m2_t)
            nc.scalar.activation(v_t, v_t, AF.Sqrt, bias=eps_t[:, :], scale=1.0)
            with nc.allow_low_precision("fp32r rstd is fp32-equivalent"):
                nc.vector.reciprocal(v_t, v_t)
            nc.vector.scalar_tensor_tensor(m_t, m_t, -1.0, v_t,
                                           op0=MULT, op1=MULT)
            return v_t, m_t

        def ln_bcast(v_t, m_t):
            a_ps = pmm.tile([P, TCH], F32, tag="mm", name="a_ps")
            nc.tensor.matmul(a_ps, _r(ones1[:, :]), v_t[:, :],
                             start=True, stop=True)
            b_ps = pmm.tile([P, TCH], F32, tag="mm", name="b_ps")
            nc.tensor.matmul(b_ps, _r(ones1[:, :]), m_t[:, :],
                             start=True, stop=True)
            return a_ps, b_ps

        def ln_apply(tsl, a_ps, b_ps, htag):
            h = []
            for cc in range(NCC):
                d = ph.tile([P, TCH], F16, tag=f"{htag}{cc}", name=f"h{cc}")
                nc.vector.tensor_mul(d, x_sb[cc][:, tsl], a_ps)
                nc.vector.tensor_add(d, d, b_ps)
                h.append(d)
            return h

        def load_weights(l):
            def _load(dram, tag, n, width):
                ts_ = []
                for i in range(n):
                    t = pw.tile([P, width], F16, tag=f"{tag}{i}", name=f"{tag}{i}")
                    nc.sync.dma_start(out=t, in_=dram[l, i * P:(i + 1) * P, :])
                    ts_.append(t)
                return ts_

            w = {}
            w["wq"] = _load(wq_d, "wq", NCC, C)
            w["wk"] = _load(wk_d, "wk", NCC, C)
            w["wv"] = _load(wv_d, "wv", NCC, C)
            w["wo"] = _load(wo_d, "wo", NCC, C)
            w["w1"] = _load(w1_d, "w1", NCC, FF)
            w["w2"] = _load(w2_d, "w2", NFF, C)
            if not zero_attn_bias:
                for nm, dr in (("bq", bq_d), ("bk", bk_d), ("bo", bo_d)):
                    t = pbias.tile([P, NCC], F32, tag=nm, name=nm)
                    nc.sync.dma_start(out=t, in_=dr[l].rearrange("(a p) -> p a", p=P))
                    w[nm] = t
                bv_bc = pbias.tile([P, C], F32, tag="bvb", name="bvb")
                nc.sync.dma_start(out=bv_bc, in_=_bcast_dram(bv_d[l], P))
                w["bv_bc"] = bv_bc
            if not zero_mlp_bias:
                t = pbias.tile([P, NFF], F32, tag="b1", name="b1")
                nc.sync.dma_start(out=t, in_=b1_d[l].rearrange("(a p) -> p a", p=P))
                w["b1"] = t
                t = pbias.tile([P, NCC], F32, tag="b2", name="b2")
                nc.sync.dma_start(out=t, in_=b2_d[l].rearrange("(a p) -> p a", p=P))
                w["b2"] = t
            return w

        def ffn2_emit(st, cc_list):
            w2_sb, ffn1, ptsl, wt_ = st
            for cc in cc_list:
                ps = pmm.tile([P, TCH], F32, tag="mm", name="mmps")
                for fc in range(NFF):
                    nc.tensor.matmul(ps, w2_sb[fc][:, cc * P:(cc + 1) * P],
                                     ffn1[fc][:, :], start=(fc == 0),
                                     stop=(fc == NFF - 1))
                if zero_mlp_bias:
                    nc.vector.tensor_add(x_sb[cc][:, ptsl], ps,
                                         x_sb[cc][:, ptsl])
                else:
                    nc.vector.scalar_tensor_tensor(
                        x_sb[cc][:, ptsl], ps, wt_["b2"][:, cc:cc + 1],
                        x_sb[cc][:, ptsl], op0=ADD, op1=ADD)

        pending = None

        for l in range(L):
            wt = load_weights(l)
            for ti in range(NTC):
                tsl = slice(ti * TCH, (ti + 1) * TCH)

                S0, S1 = ln_stats(tsl)
                ABl = ln_finish(S0, S1)
                if pending is not None:
                    ffn2_emit(pending, [0, 1])
                a_ps, b_ps = ln_bcast(*ABl)
                if pending is not None:
                    ffn2_emit(pending, [2, 3])
                    pending = None
                h1 = ln_apply(tsl, a_ps, b_ps, "h")

                q_t, k_t = [], []
                for dst, wsb, bnm in ((q_t, wt["wq"], "bq"),
                                      (k_t, wt["wk"], "bk")):
                    for hq in range(NCC):
                        ps = pmm.tile([P, TCH], F32, tag="mm", name="mmps")
                        for cc in range(NCC):
                            nc.tensor.matmul(ps, wsb[cc][:, hq * P:(hq + 1) * P],
                                             h1[cc][:, :], start=(cc == 0),
                                             stop=(cc == NCC - 1))
                        qt = pq.tile([P, TCH], F16, tag=f"{bnm}t{hq}",
                                     name=f"{bnm}t{hq}")
                        if zero_attn_bias:
                            nc.scalar.copy(qt, ps)
                        else:
                            nc.scalar.activation(qt, ps, AF.Identity,
                                                 bias=wt[bnm][:, hq:hq + 1],
                                                 scale=1.0)
                        dst.append(qt)
                v8 = []
                for tt in range(TCH // P):
                    ps = pmm.tile([P, C], F32, tag="mm", name="mmps")
                    for cc in range(NCC):
                        nc.tensor.matmul(ps, h1[cc][:, tt * P:(tt + 1) * P],
                                         wt["wv"][cc][:, :], start=(cc == 0),
                                         stop=(cc == NCC - 1))
                    vt = pv.tile([P, H, HD], F16, tag=f"v{tt}", name=f"vt{tt}")
                    if zero_attn_bias:
                        nc.scalar.copy(vt, ps[:].rearrange("p (h d) -> p h d", h=H))
                    else:
                        nc.vector.tensor_add(
                            vt, ps[:].rearrange("p (h d) -> p h d", h=H),
                            wt["bv_bc"][:].rearrange("p (h d) -> p h d", h=H))
                    v8.append(vt)

                o_t = [po.tile([P, TCH], F16, tag=f"o{hq}", name=f"ot{hq}")
                       for hq in range(NCC)]
                for bi in range(BI):
                    den_ps = pmm.tile([H, T], F32, tag="mm", name="den_ps")
                    for hh in range(H):
                        hq, hr = divmod(hh, 2)
                        rsl = slice(hr * HD, (hr + 1) * HD)
                        qsl = q_t[hq][rsl, bi * T:(bi + 1) * T]
                        ksl0 = k_t[hq][rsl, bi * T: bi * T + P]
                        ksl1 = k_t[hq][rsl, bi * T + P: bi * T + 2 * P]
                        sc_ps = psc.tile([P, 2, T], F32, tag="scps", name="scps")
                        nc.tensor.matmul(sc_ps[:, 0, :], ksl0, qsl,
                                         start=True, stop=True)
                        nc.tensor.matmul(sc_ps[:, 1, P:T], ksl1, qsl[:, P:T],
                                         start=True, stop=True)
                        e = pe_.tile([P, 2, T], F16, tag="e", name="e")
                        nc.scalar.activation(e[:, 0, :], sc_ps[:, 0, :], AF.Exp)
                        nc.scalar.activation(e[:, 1, P:T], sc_ps[:, 1, P:T],
                                             AF.Exp)
                        nc.gpsimd.tensor_mul(e[:, 0, :], e[:, 0, :],
                                             mask_sb[:, 0, :])
                        nc.gpsimd.tensor_mul(e[:, 1, P:T], e[:, 1, P:T],
                                             mask_sb[:, 1, P:T])
                        o_ps = pops.tile([HD, T], F32, tag="ops", name="ops")
                        nc.tensor.matmul(o_ps, v8[bi * 2][:, hh, :], e[:, 0, :],
                                         start=True, stop=False)
                        nc.tensor.matmul(o_ps[:, P:T], v8[bi * 2 + 1][:, hh, :],
                                         e[:, 1, P:T], start=False, stop=True)
                        nc.tensor.matmul(den_ps, o8c[:, hh, :], e[:, 0, :],
                                         start=(hh == 0), stop=False)
                        nc.tensor.matmul(den_ps[:, P:T], o8c[:, hh, :],
                                         e[:, 1, P:T], start=False,
                                         stop=(hh == H - 1))
                        nc.scalar.copy(o_t[hq][rsl, bi * T:(bi + 1) * T],
                                       o_ps[0:HD, :])
                    rden = prd.tile([H, T], F32R, tag="rden", name="rden")
                    with nc.allow_low_precision("fp32r rden is fp32-equivalent"):
                        nc.vector.reciprocal(rden, den_ps)
                    for hq in range(NCC):
                        rdb = pmm.tile([P, T], F32, tag="mm", name="rdb")
                        nc.tensor.matmul(rdb, e8[hq][:, :], rden[:, :],
                                         start=True, stop=True)
                        osl = o_t[hq][:, bi * T:(bi + 1) * T]
                        nc.vector.tensor_mul(osl, osl, rdb)

                for cc in range(NCC):
                    ps = pmm.tile([P, TCH], F32, tag="mm", name="mmps")
                    for hq in range(NCC):
                        nc.tensor.matmul(ps, wt["wo"][hq][:, cc * P:(cc + 1) * P],
                                         o_t[hq][:, :], start=(hq == 0),
                                         stop=(hq == NCC - 1))
                    if zero_attn_bias:
                        nc.vector.tensor_add(x_sb[cc][:, tsl], ps,
                                             x_sb[cc][:, tsl])
                    else:
                        nc.vector.scalar_tensor_tensor(
                            x_sb[cc][:, tsl], ps, wt["bo"][:, cc:cc + 1],
                            x_sb[cc][:, tsl], op0=ADD, op1=ADD)

                S0, S1 = ln_stats(tsl)
                ABl = ln_finish(S0, S1)
                a_ps, b_ps = ln_bcast(*ABl)
                h2 = ln_apply(tsl, a_ps, b_ps, "g")
                ffn1 = []
                for fc in range(NFF):
                    ps = pmm.tile([P, TCH], F32, tag="mm", name="mmps")
                    for cc in range(NCC):
                        nc.tensor.matmul(ps, wt["w1"][cc][:, fc * P:(fc + 1) * P],
                                         h2[cc][:, :], start=(cc == 0),
                                         stop=(cc == NCC - 1))
                    ft = pffn.tile([P, TCH], F16, tag=f"f{fc}", name=f"ft{fc}")
                    if zero_mlp_bias:
                        nc.scalar.activation(ft, ps, AF.Relu)
                    else:
                        nc.scalar.activation(ft, ps, AF.Relu,
                                             bias=wt["b1"][:, fc:fc + 1],
                                             scale=1.0)
                    ffn1.append(ft)
                pending = (wt["w2"], ffn1, tsl, wt)

        ffn2_emit(pending, [0, 1, 2, 3])
        pending = None

        for tt in range(NT // P):
            ps = pmm.tile([P, V], F32, tag="mm", name="mmps")
            for cc in range(NCC):
                nc.tensor.matmul(ps, x_sb[cc][:, tt * P:(tt + 1) * P].bitcast(F32),
                                 wlm_sb[cc][:, :], start=(cc == 0),
                                 stop=(cc == NCC - 1))
            lo = plog.tile([P, V], F32, tag="lg", name="lo")
            nc.vector.tensor_add(lo, ps, blm_bc)
            nc.sync.dma_start(out=out_d[tt * P:(tt + 1) * P, :], in_=lo)

    if not nc.is_finalized():
        nc.finalize()
    return nc


def prep_inputs(idx, tok_emb, pos_emb, Wq, Wk, Wv, Wo, bo, ln1_g, ln1_b,
                ln2_g, ln2_b, W1, b1, W2, b2, Wlm, blm):
    """host-side: fold LN affines into weights, build per-core input maps"""
    f32 = np.float32
    idx = np.asarray(idx)
    tok_emb = np.asarray(tok_emb, f32)
    pos_emb = np.asarray(pos_emb, f32)
    scale = C ** -0.5

    wq = np.empty((L, C, C), f32)
    wk = np.empty((L, C, C), f32)
    wv = np.empty((L, C, C), f32)
    wo = np.empty((L, C, C), f32)
    w1 = np.empty((L, C, FF), f32)
    w2 = np.empty((L, FF, C), f32)
    bq = np.empty((L, C), f32)
    bk = np.empty((L, C), f32)
    bv = np.empty((L, C), f32)
    b1f = np.empty((L, FF), f32)
    for l in range(L):
        wq_c = np.asarray(Wq[l], f32).transpose(1, 0, 2).reshape(C, C)
        wk_c = np.asarray(Wk[l], f32).transpose(1, 0, 2).reshape(C, C)
        wv_c = np.asarray(Wv[l], f32).transpose(1, 0, 2).reshape(C, C)
        g1 = np.asarray(ln1_g[l], f32)[:, None]
        b1_ = np.asarray(ln1_b[l], f32)
        g2 = np.asarray(ln2_g[l], f32)[:, None]
        b2_ = np.asarray(ln2_b[l], f32)
        wq[l] = g1 * wq_c * scale
        bq[l] = (b1_ @ wq_c) * scale
        wk[l] = g1 * wk_c
        bk[l] = b1_ @ wk_c
        wv[l] = g1 * wv_c
        bv[l] = b1_ @ wv_c
        wo[l] = np.asarray(Wo[l], f32)
        w1[l] = g2 * np.asarray(W1[l], f32)
        b1f[l] = np.asarray(b1[l], f32) + b2_ @ np.asarray(W1[l], f32)
        w2[l] = np.asarray(W2[l], f32)

    bo = np.asarray(bo, f32)
    b2a = np.asarray(b2, f32)

    # legacy full mask in [s%128, s//128, t] layout
    s_g = np.arange(2 * P).reshape(2, P).T
    mask = (s_g[:, :, None] <= np.arange(T)[None, None, :]).astype(np.float16)
    # v2 mask: one diagonal [P,P] triangular block (s_local <= t_local)
    tri = (np.arange(P)[:, None] <= np.arange(P)[None, :]).astype(np.float16)

    pos2 = np.concatenate([pos_emb.T, pos_emb.T], axis=1)  # [C, 512]

    flags = {
        "zero_attn_bias": not (np.any(bq) or np.any(bk) or np.any(bv)
                               or np.any(bo)),
        "zero_mlp_bias": not (np.any(b1f) or np.any(b2a)),
        "zero_lm_bias": not np.any(np.asarray(blm, f32)),
    }

    e8sel = np.zeros((NCC, H, P), f32)
    for hq in range(NCC):
        for p_ in range(P):
            e8sel[hq, 2 * hq + p_ // HD, p_] = 1.0

    shared = {
        "e8sel": e8sel,
        "tok_emb16": tok_emb.astype(np.float16),
        "pos2T": np.ascontiguousarray(pos2, f32),
        "wq": wq.astype(np.float16), "wk": wk.astype(np.float16),
        "wv": wv.astype(np.float16), "wo": wo.astype(np.float16),
        "w1": w1.astype(np.float16), "w2": w2.astype(np.float16),
        "wlm": np.asarray(Wlm, f32),
    }
    if all(flags.values()):
        shared["triT"] = np.ascontiguousarray(tri)
        shared["pos2T16"] = np.ascontiguousarray(pos2).astype(np.float16)
        del shared["pos2T"]
    else:
        shared["maskT"] = np.ascontiguousarray(mask)
        shared["bq"] = bq
        shared["bk"] = bk
        shared["bv"] = bv
        shared["bo"] = bo
        shared["b1"] = b1f
        shared["b2"] = b2a
        shared["blm"] = np.asarray(blm, f32)
    in_maps = []
    vocab = np.arange(V)
    for core in range(NCORES):
        toks = np.asarray(idx[core * BL:(core + 1) * BL]).reshape(-1)
        oh = (vocab[:, None] == toks[None, :]).astype(np.float16)
        m = dict(shared)
        m["onehotT"] = np.ascontiguousarray(oh)
        in_maps.append(m)
    return in_maps, flags


_NC_CACHE = {}


def get_nc(flags=None):
    if flags is None:
        flags = {"zero_attn_bias": True, "zero_mlp_bias": True,
                 "zero_lm_bias": True}
    if all(flags.get(k, False) for k in
           ("zero_attn_bias", "zero_mlp_bias", "zero_lm_bias")):
        key = "v2"
        if key not in _NC_CACHE:
            _NC_CACHE[key] = build_bass_v2()
    else:
        key = (flags["zero_attn_bias"], flags["zero_mlp_bias"])
        if key not in _NC_CACHE:
            _NC_CACHE[key] = build_bass_legacy(
                zero_attn_bias=flags["zero_attn_bias"],
                zero_mlp_bias=flags["zero_mlp_bias"])
    return _NC_CACHE[key]


def run(in_maps, flags=None, trace=False, **kw):
    from concourse.bass_utils import run_bass_kernel_spmd
    nc = get_nc(flags)
    return run_bass_kernel_spmd(nc, in_maps, list(range(NCORES)),
                                trace=trace, **kw)


def kernel(**inputs):
    in_maps, flags = prep_inputs(**inputs)
    res = run(in_maps, flags)
    outs = []
    for i in range(NCORES):
        r = res.results[i]
        if "outT" in r:
            outs.append(np.ascontiguousarray(r["outT"].T).reshape(BL, T, V))
        else:
            outs.append(r["out"].reshape(BL, T, V))
    return np.concatenate(outs, axis=0).astype(np.float32)
